# revision 46
# baseline (speedup 1.0000x reference)
"""Time-varying 33-tap FIR low-pass filter on 8 Trainium2 NeuronCores.

y[b,t] = sum_u filt[t,u] * x[b, t+u-16],  filt = host-computed windowed-sinc
bank (n,33) derived from scalars alpha/beta (tiny; O(n*33) host FLOPs).

Sharding: time dim split across the 8 cores (4096 t-columns each, all 64
batch rows).  Per core the banded matmul y = x @ W (contraction over input
time s) is tiled into 22 TensorE matmuls.  Each matmul packs TWO 128-sample
x-chunks, offset by 96 samples, side by side in the stationary operand
(K=128, M=128 = 2 halves x 64 batch).  The 96-offset makes every output
column's 33-tap band land entirely inside one half, so each PSUM column is
valid in exactly one 64-row half and the chunk serves 192 output columns
with no accumulation pass:

  lhsT[k, 64*h + b] = x[b, S + 96*h + k]           (S = core_t0 - 16 + 192*j)
  rhs [k, n]        = filt[S+16+n, u] at k = (n % 96) + u   (zeros elsewhere)
  psum[64*h(n) + b, n] = y[b, S+16+n],   h(n) = n // 96

Performance notes (38.9us baseline -> ~22us):
- all matmul operands bf16 (fp32 matmul runs 4 cycles/row on TRN2, bf16
  runs 1; input DMA bytes halve; rel err ~6e-3, gate is 2e-2)
- input laid out partition-major ([128, NJ*W]) and moved by a few LARGE
  grouped DMAs: the 16 per-core DMA engines have ~30ns/packet overhead, so
  long lines (group_chunks*640B) sustain ~280-306GB/s aggregate
- valid PSUM half-rows are extracted on-device -- DVE copies half0 (cols
  0:96 from partitions 0:64), the Activation engine copies half1 (cols
  96:192 from partitions 64:128) -- converting to bf16 into a compact
  [128, NJ*96] staging tile (half1 stays in partitions 64:128 so output
  DMAs move 128 lines, not 64), so the output DMA moves 1/4 of the
  baseline bytes; host unfolds the halves and upcasts to fp32
- extraction is batched via multi-bank 3D PSUM access patterns: quads up
  front for batching efficiency, pairs near the end so the tail's
  PSUM-reuse and output-DMA links are short
- output DMAs are queued just-in-time after the input stream drains (they
  share the DMA engines), split between gpsimd's SWDGE and the idle Sync
  engine's HWDGE so descriptor generations overlap
- the unused const-AP publish barrier in Bass.__init__ is skipped (~0.3us)
- no explicit semaphore-clear block: the nc.semaphore() context managers
  already emit clears at ExitStack exit behind the full-drain exit barrier
  (verified safe across NEFF re-executions), saving ~1us of teardown
Fixed costs bound further gains: ~6.7us NEFF preamble (runtime rendezvous,
iram loads, engine preambles), ~0.9us per-DMA completion-semaphore
latency, ~2us teardown (drains, exit barrier, scope-exit semaphore
clears).  An empty NEFF measures ~11.1us on this runtime.
"""

import sys
from contextlib import ExitStack

import numpy as np
import ml_dtypes

if "/opt/trn_rl_repo" not in sys.path:
    sys.path.insert(0, "/opt/trn_rl_repo")

from concourse import bass, mybir
from concourse.bass_utils import run_bass_kernel_spmd

N = 32768          # time length
B = 64             # batch
NCORES = 8
TCORE = N // NCORES            # 4096 output columns per core
CT = 192                       # output columns served per chunk
NJ = (TCORE + CT - 1) // CT    # 22 chunks per core (last one partial: 64 cols)
KP = 128                       # contraction rows per matmul
TAPS = 33
HALF = 16
W = 128 + CT                   # 320 columns per [stationary | moving] chunk

# input DMA groups (chunk boundaries).  Few LARGE groups: the 16 DMA engines
# run ~15-20GB/s per engine with ~30ns/packet overhead, so longer lines
# (group_chunks*640B) raise the per-engine rate.  A tiny first group starts
# compute early; a small last group keeps the final matmuls off a big
# trailing transfer's completion latency
IN_GROUPS = (4, 12, 17, 20, NJ)
# extraction units: quads up front (efficient batching), pairs from chunk 12
# on so the tail's PSUM-reuse and output-issue links are short
EX_UNITS = ((0, 4), (4, 8), (8, 12), (12, 14), (14, 16), (16, 18), (18, 20), (20, NJ))
# output DMA groups as extraction-unit thresholds -> chunk ranges
# [0:4) [4:12) [12:16) [16:20) [20:22).  Queued just-in-time: transfers share
# the 16 DMA engines with the input stream, so the first (small) group rides
# right behind the input stream's tail and the rest chase extraction.  All
# but the last two go through gpsimd's SWDGE; the last two are issued by the
# (idle) Sync engine's HWDGE so their ~0.6us descriptor generations overlap
# gpsimd's instead of serializing.
OUT_GROUPS_U = (1, 3, 5, 7, 8)

_prog_cache = None


def _filters_np(alpha, beta):
    """Numpy port of reference._filters (returns the flipped bank)."""
    t = np.arange(N, dtype=np.float64)
    cutoff = (np.pi / 4.0 + float(alpha) * np.sin(float(beta) * t / 8000.0)) / (
        2.0 * np.pi
    )
    k = np.arange(TAPS, dtype=np.float64)
    window = 0.5 - 0.5 * np.cos(2.0 * np.pi * k / (TAPS - 1.0))
    tvec = np.arange(-HALF, HALF + 1, dtype=np.float64)
    arg = 2.0 * np.pi * cutoff[:, None] * tvec[None, :]
    safe = np.where(arg == 0.0, 1.0, arg)
    sinc = np.where(arg == 0.0, 1.0, np.sin(safe) / safe)
    f = 2.0 * cutoff[:, None] * window[None, :] * sinc
    f = f / f.sum(axis=-1, keepdims=True)
    return np.ascontiguousarray(f[:, ::-1]).astype(np.float32)


def _prep_inputs(x, alpha, beta):
    """Build per-core [KP, NJ*W] bf16 [stationary | banded-filter] tiles."""
    filt = _filters_np(alpha, beta)  # (N, 33)

    pad = 16 + N + 512
    xp = np.zeros((B, pad), dtype=np.float32)
    xp[:, 16 : 16 + N] = x
    xp = xp.astype(ml_dtypes.bfloat16)
    fp = np.zeros((N + 512, TAPS), dtype=ml_dtypes.bfloat16)
    fp[:N] = filt.astype(ml_dtypes.bfloat16)

    c = np.arange(NCORES)[:, None, None, None]
    j = np.arange(NJ)[None, :, None, None]
    h = np.arange(2)[None, None, :, None]
    k = np.arange(KP)[None, None, None, :]
    # global s = TCORE*c - 16 + CT*j + 96*h + k ; +16 shifts into xp coords
    sidx = TCORE * c + CT * j + 96 * h + k
    xw = xp[:, sidx]  # (B, NCORES, NJ, 2, KP)
    xw = np.ascontiguousarray(
        np.transpose(xw, (1, 2, 4, 3, 0)).reshape(NCORES, NJ, KP, 128)
    )

    u = np.arange(TAPS)[:, None]  # (33, 1)
    nn = np.arange(CT)[None, :]  # (1, 192)
    rows = (nn % 96) + u  # (33, 192) target partition rows
    cols = np.broadcast_to(nn, (TAPS, CT))
    tg = (
        TCORE * np.arange(NCORES)[:, None, None]
        + CT * np.arange(NJ)[None, :, None]
        + np.arange(CT)[None, None, :]
    )  # (NCORES, NJ, 192) global output t per column
    vals = np.transpose(fp[tg], (0, 1, 3, 2))  # (NCORES, NJ, 33, 192)
    wt = np.zeros((NCORES, NJ, KP, CT), dtype=ml_dtypes.bfloat16)
    wt[:, :, rows, cols] = vals

    # one combined [stationary | moving] tile per chunk, then partition-major
    # ([KP, NJ, W]) so grouped input DMAs move long contiguous lines
    xwt = np.concatenate([xw, wt], axis=3)  # (NCORES, NJ, KP, W)
    xwt = np.transpose(xwt, (0, 2, 1, 3)).reshape(NCORES, KP, NJ * W)
    return np.ascontiguousarray(xwt)


def _build_program():
    """Raw Bass (no Tile): walrus permits a single sync-wait slot per Matmult
    and per DMA descriptor, so waits are emitted as standalone EventSemaphore
    instructions on each engine's queue instead."""
    # Skip the const-AP publish barrier at the tail of Bass.__init__: this
    # kernel never reads const_aps (scalar Copy keeps a float bias), the NRT
    # pseudo-barrier earlier in init already rendezvoused the engines, and
    # per-sem waits order everything else.  Saves ~0.3us of preamble.
    orig_aeb = bass.Bass.all_engine_barrier
    bass.Bass.all_engine_barrier = lambda self, *, sem_only=False: None
    try:
        nc = bass.Bass(trn_type="TRN2", debug=False)
    finally:
        bass.Bass.all_engine_barrier = orig_aeb
    f32 = mybir.dt.float32
    bf16 = mybir.dt.bfloat16
    xwt_d = nc.dram_tensor("xwt", [KP, NJ * W], bf16, kind="ExternalInput").ap()
    y_d = nc.dram_tensor("y", [2 * B, NJ * 96], bf16, kind="ExternalOutput").ap()

    def grp(j):
        """Input-group index of chunk j."""
        for g, gend in enumerate(IN_GROUPS):
            if j < gend:
                return g

    with ExitStack() as ctx:
        xts = ctx.enter_context(nc.sbuf_tensor("xts", [KP, NJ * W], bf16))
        # staging keeps half1 in partitions 64:128 (no partition fold), so
        # output DMAs move 128 lines instead of 64 -- better engine pipelining
        st = ctx.enter_context(nc.sbuf_tensor("st", [2 * B, NJ, 96], bf16))
        # 2 quad-tensors of 4 PSUM banks each (slot = 512 fp32 = one bank);
        # extraction reads all slots of a quad in one 3D-AP op
        pps = [
            ctx.enter_context(nc.psum_tensor(f"pp{i}", [128, 4, 512], f32))
            for i in range(2)
        ]
        # one semaphore per input DMA (no semaphore reuse -> no guards needed)
        sem_in = [
            ctx.enter_context(nc.semaphore(f"s_in{i}"))
            for i in range(len(IN_GROUPS))
        ]
        sem_pe = ctx.enter_context(nc.semaphore("s_pe"))
        sem_dve = ctx.enter_context(nc.semaphore("s_dve"))
        sem_act = ctx.enter_context(nc.semaphore("s_act"))
        # qPoolDynamic is a single FIFO queue, so one semaphore (+16 per DMA)
        # suffices for the gpsimd-issued groups; the sync-issued groups land
        # on distinct qSPDynamicHW queues and get their own semaphores
        sem_out = ctx.enter_context(nc.semaphore("s_out"))
        sem_out2 = [
            ctx.enter_context(nc.semaphore(f"s_out2{i}")) for i in range(2)
        ]
        block_cm = nc.Block()
        block = block_cm.__enter__()

        # chunk ranges of the output groups (unit thresholds -> chunk bounds)
        out_bounds = [0] + [EX_UNITS[u - 1][1] for u in OUT_GROUPS_U]

        in_bounds = [0] + list(IN_GROUPS)

        @block.sync
        def _(sync):
            # all input groups from one engine: generation order == transfer
            # order, so group 0 always reaches the DMA engines first
            for g in range(len(IN_GROUPS)):
                sync.dma_start(
                    out=xts[:, W * in_bounds[g] : W * in_bounds[g + 1]],
                    in_=xwt_d[:, W * in_bounds[g] : W * in_bounds[g + 1]],
                ).then_inc(sem_in[g], 16)
            # last two output groups: HWDGE generation here overlaps gpsimd's
            for i, gi in enumerate((len(OUT_GROUPS_U) - 2, len(OUT_GROUPS_U) - 1)):
                u = OUT_GROUPS_U[gi]
                c0, c1 = out_bounds[gi], out_bounds[gi + 1]
                sync.wait_ge(sem_dve, u)
                sync.wait_ge(sem_act, u)
                sync.dma_start(
                    out=y_d[:, 96 * c0 : 96 * c1], in_=st[:, c0:c1, :]
                ).then_inc(sem_out2[i], 16)

        def unit_of(c):
            for u, (c0, c1) in enumerate(EX_UNITS):
                if c0 <= c < c1:
                    return u

        @block.tensor
        def _(tensor):
            reuse_th = 0
            for j in range(NJ):
                g = grp(j)
                if j == 0 or grp(j - 1) != g:
                    tensor.wait_ge(sem_in[g], 16)
                if j >= 8:
                    # PSUM slot of chunk j (bank j%8) is free once the unit
                    # holding chunk j-8 has BOTH half-copies retired
                    th = unit_of(j - 8) + 1
                    if th > reuse_th:
                        tensor.wait_ge(sem_dve, th)
                        tensor.wait_ge(sem_act, th)
                        reuse_th = th
                tensor.matmul(
                    pps[(j // 4) % 2].ap()[:, j % 4, 0:CT],
                    xts[:, W * j : W * j + 128],
                    xts[:, W * j + 128 : W * (j + 1)],
                    start=True,
                    stop=True,
                ).then_inc(sem_pe, 1)

        @block.vector
        def _(vector):
            # half0: outputs 0:96 of each chunk live in PSUM partitions 0:64;
            # one 3D-AP op extracts a whole unit (slots c0%4..)
            for u, (c0, c1) in enumerate(EX_UNITS):
                vector.wait_ge(sem_pe, c1)
                vector.tensor_copy(
                    st[0:B, c0:c1, 0:96],
                    pps[(c0 // 4) % 2].ap()[0:B, c0 % 4 : c0 % 4 + c1 - c0, 0:96],
                ).then_inc(sem_dve, 1)

        @block.scalar
        def _(scalar):
            # half1: outputs 96:192 of each chunk live in PSUM partitions 64:128
            for u, (c0, c1) in enumerate(EX_UNITS):
                scalar.wait_ge(sem_pe, c1)
                scalar.copy(
                    st[B : 2 * B, c0:c1, 0:96],
                    pps[(c0 // 4) % 2].ap()[B : 2 * B, c0 % 4 : c0 % 4 + c1 - c0, 96:CT],
                ).then_inc(sem_act, 1)

        @block.gpsimd
        def _(gpsimd):
            for gi in range(len(OUT_GROUPS_U) - 2):
                u = OUT_GROUPS_U[gi]
                c0, c1 = out_bounds[gi], out_bounds[gi + 1]
                gpsimd.wait_ge(sem_dve, u)
                gpsimd.wait_ge(sem_act, u)
                gpsimd.dma_start(
                    out=y_d[:, 96 * c0 : 96 * c1], in_=st[:, c0:c1, :]
                ).then_inc(sem_out, 16)
            gpsimd.wait_ge(sem_out, 16 * (len(OUT_GROUPS_U) - 2))
            for s in sem_out2:
                gpsimd.wait_ge(s, 16)

        block_cm.__exit__(None, None, None)  # all-engine exit barrier
        # no explicit semaphore-clear block: the nc.semaphore() context
        # managers emit clears for every sem at ExitStack exit, behind the
        # full-drain barrier above, so re-executing the NEFF is clean

    return nc


def run_sharded(inputs, trace=False):
    global _prog_cache
    x = np.ascontiguousarray(np.asarray(inputs["input"], dtype=np.float32))
    xwt = _prep_inputs(x, inputs["alpha"], inputs["beta"])
    if _prog_cache is None:
        _prog_cache = _build_program()
    nc = _prog_cache
    in_maps = [{"xwt": xwt[cc]} for cc in range(NCORES)]
    res = run_bass_kernel_spmd(nc, in_maps, list(range(NCORES)), trace=trace)
    shards = []
    for cc in range(NCORES):
        yd = res.results[cc]["y"].reshape(2, B, NJ, 96)
        yc = np.transpose(yd, (1, 2, 0, 3)).reshape(B, NJ * CT)
        shards.append(yc[:, :TCORE].astype(np.float32))
    y = np.concatenate(shards, axis=1)
    return y, res


def kernel(input, alpha, beta):
    y, _ = run_sharded({"input": input, "alpha": alpha, "beta": beta})
    return y


# revision 47
# speedup vs baseline: 1.1251x; 1.1251x over previous
"""Time-varying 33-tap FIR low-pass filter on 8 Trainium2 NeuronCores.

y[b,t] = sum_u filt[t,u] * x[b, t+u-16],  filt = host-computed windowed-sinc
bank (n,33) derived from scalars alpha/beta (tiny; O(n*33) host FLOPs).

Sharding: time dim split across the 8 cores (4096 t-columns each, all 64
batch rows).  Per core the banded matmul y = x @ W (contraction over input
time s) is tiled into 22 TensorE matmuls.  Each matmul packs TWO 128-sample
x-chunks, offset by 96 samples, side by side in the stationary operand
(K=128, M=128 = 2 halves x 64 batch).  The 96-offset makes every output
column's 33-tap band land entirely inside one half, so each PSUM column is
valid in exactly one 64-row half and the chunk serves 192 output columns
with no accumulation pass:

  lhsT[k, 64*h + b] = x[b, S + 96*h + k]           (S = core_t0 - 16 + 192*j)
  rhs [k, n]        = filt[S+16+n, u] at k = (n % 96) + u   (zeros elsewhere)
  psum[64*h(n) + b, n] = y[b, S+16+n],   h(n) = n // 96

Performance notes (38.9us baseline -> ~22us):
- all matmul operands bf16 (fp32 matmul runs 4 cycles/row on TRN2, bf16
  runs 1; input DMA bytes halve; rel err ~6e-3, gate is 2e-2)
- input laid out partition-major ([128, NJ*W]) and moved by a few LARGE
  grouped DMAs: the 16 per-core DMA engines have ~30ns/packet overhead, so
  long lines (group_chunks*640B) sustain ~280-306GB/s aggregate
- valid PSUM half-rows are extracted on-device -- DVE copies half0 (cols
  0:96 from partitions 0:64), the Activation engine copies half1 (cols
  96:192 from partitions 64:128) -- converting to bf16 into a compact
  [128, NJ*96] staging tile (half1 stays in partitions 64:128 so output
  DMAs move 128 lines, not 64), so the output DMA moves 1/4 of the
  baseline bytes; host unfolds the halves and upcasts to fp32
- extraction is batched via multi-bank 3D PSUM access patterns: quads up
  front for batching efficiency, pairs near the end so the tail's
  PSUM-reuse and output-DMA links are short
- output DMAs are queued just-in-time after the input stream drains (they
  share the DMA engines), split between gpsimd's SWDGE and the idle Sync
  engine's HWDGE so descriptor generations overlap
- the unused const-AP publish barrier in Bass.__init__ is skipped (~0.3us)
- no explicit semaphore-clear block: the nc.semaphore() context managers
  already emit clears at ExitStack exit behind the full-drain exit barrier
  (verified safe across NEFF re-executions), saving ~1us of teardown
Fixed costs bound further gains: ~6.7us NEFF preamble (runtime rendezvous,
iram loads, engine preambles), ~0.9us per-DMA completion-semaphore
latency, ~2us teardown (drains, exit barrier, scope-exit semaphore
clears).  An empty NEFF measures ~11.1us on this runtime.
"""

import sys
from contextlib import ExitStack

import numpy as np
import ml_dtypes

if "/opt/trn_rl_repo" not in sys.path:
    sys.path.insert(0, "/opt/trn_rl_repo")

from concourse import bass, mybir
from concourse.bass_utils import run_bass_kernel_spmd

N = 32768          # time length
B = 64             # batch
NCORES = 8
TCORE = N // NCORES            # 4096 output columns per core
CT = 192                       # output columns served per chunk
NJ = (TCORE + CT - 1) // CT    # 22 chunks per core (last one partial: 64 cols)
KP = 128                       # contraction rows per matmul
TAPS = 33
HALF = 16
W = 128 + CT                   # 320 columns per [stationary | moving] chunk

# input DMA groups (chunk boundaries).  Few LARGE groups: the 16 DMA engines
# run ~15-20GB/s per engine with ~30ns/packet overhead, so longer lines
# (group_chunks*640B) raise the per-engine rate.  A tiny first group starts
# compute early; a small last group keeps the final matmuls off a big
# trailing transfer's completion latency
IN_GROUPS = (4, 12, 18, NJ)
# extraction units: quads up front (efficient batching), pairs from chunk 12
# on so the tail's PSUM-reuse and output-issue links are short
EX_UNITS = ((0, 4), (4, 8), (8, 12), (12, 14), (14, 16), (16, 18), (18, 20), (20, NJ))
# output DMA groups as extraction-unit thresholds -> chunk ranges
# [0:4) [4:12) [12:16) [16:20) [20:22).  Queued just-in-time: transfers share
# the 16 DMA engines with the input stream, so the first (small) group rides
# right behind the input stream's tail and the rest chase extraction.  All
# but the last two go through gpsimd's SWDGE; the last two are issued by the
# (idle) Sync engine's HWDGE so their ~0.6us descriptor generations overlap
# gpsimd's instead of serializing.
OUT_GROUPS_U = (1, 3, 5, 7, 8)

_prog_cache = None


def _filters_np(alpha, beta):
    """Numpy port of reference._filters (returns the flipped bank)."""
    t = np.arange(N, dtype=np.float64)
    cutoff = (np.pi / 4.0 + float(alpha) * np.sin(float(beta) * t / 8000.0)) / (
        2.0 * np.pi
    )
    k = np.arange(TAPS, dtype=np.float64)
    window = 0.5 - 0.5 * np.cos(2.0 * np.pi * k / (TAPS - 1.0))
    tvec = np.arange(-HALF, HALF + 1, dtype=np.float64)
    arg = 2.0 * np.pi * cutoff[:, None] * tvec[None, :]
    safe = np.where(arg == 0.0, 1.0, arg)
    sinc = np.where(arg == 0.0, 1.0, np.sin(safe) / safe)
    f = 2.0 * cutoff[:, None] * window[None, :] * sinc
    f = f / f.sum(axis=-1, keepdims=True)
    return np.ascontiguousarray(f[:, ::-1]).astype(np.float32)


def _prep_inputs(x, alpha, beta):
    """Build per-core [KP, NJ*W] bf16 [stationary | banded-filter] tiles."""
    filt = _filters_np(alpha, beta)  # (N, 33)

    pad = 16 + N + 512
    xp = np.zeros((B, pad), dtype=np.float32)
    xp[:, 16 : 16 + N] = x
    xp = xp.astype(ml_dtypes.bfloat16)
    fp = np.zeros((N + 512, TAPS), dtype=ml_dtypes.bfloat16)
    fp[:N] = filt.astype(ml_dtypes.bfloat16)

    c = np.arange(NCORES)[:, None, None, None]
    j = np.arange(NJ)[None, :, None, None]
    h = np.arange(2)[None, None, :, None]
    k = np.arange(KP)[None, None, None, :]
    # global s = TCORE*c - 16 + CT*j + 96*h + k ; +16 shifts into xp coords
    sidx = TCORE * c + CT * j + 96 * h + k
    xw = xp[:, sidx]  # (B, NCORES, NJ, 2, KP)
    xw = np.ascontiguousarray(
        np.transpose(xw, (1, 2, 4, 3, 0)).reshape(NCORES, NJ, KP, 128)
    )

    u = np.arange(TAPS)[:, None]  # (33, 1)
    nn = np.arange(CT)[None, :]  # (1, 192)
    rows = (nn % 96) + u  # (33, 192) target partition rows
    cols = np.broadcast_to(nn, (TAPS, CT))
    tg = (
        TCORE * np.arange(NCORES)[:, None, None]
        + CT * np.arange(NJ)[None, :, None]
        + np.arange(CT)[None, None, :]
    )  # (NCORES, NJ, 192) global output t per column
    vals = np.transpose(fp[tg], (0, 1, 3, 2))  # (NCORES, NJ, 33, 192)
    wt = np.zeros((NCORES, NJ, KP, CT), dtype=ml_dtypes.bfloat16)
    wt[:, :, rows, cols] = vals

    # one combined [stationary | moving] tile per chunk, then partition-major
    # ([KP, NJ, W]) so grouped input DMAs move long contiguous lines
    xwt = np.concatenate([xw, wt], axis=3)  # (NCORES, NJ, KP, W)
    xwt = np.transpose(xwt, (0, 2, 1, 3)).reshape(NCORES, KP, NJ * W)
    return np.ascontiguousarray(xwt)


def _build_program():
    """Raw Bass (no Tile): walrus permits a single sync-wait slot per Matmult
    and per DMA descriptor, so waits are emitted as standalone EventSemaphore
    instructions on each engine's queue instead."""
    # Skip the const-AP publish barrier at the tail of Bass.__init__: this
    # kernel never reads const_aps (scalar Copy keeps a float bias), the NRT
    # pseudo-barrier earlier in init already rendezvoused the engines, and
    # per-sem waits order everything else.  Saves ~0.3us of preamble.
    orig_aeb = bass.Bass.all_engine_barrier
    bass.Bass.all_engine_barrier = lambda self, *, sem_only=False: None
    try:
        nc = bass.Bass(trn_type="TRN2", debug=False)
    finally:
        bass.Bass.all_engine_barrier = orig_aeb
    f32 = mybir.dt.float32
    bf16 = mybir.dt.bfloat16
    xwt_d = nc.dram_tensor("xwt", [KP, NJ * W], bf16, kind="ExternalInput").ap()
    y_d = nc.dram_tensor("y", [2 * B, NJ * 96], bf16, kind="ExternalOutput").ap()

    def grp(j):
        """Input-group index of chunk j."""
        for g, gend in enumerate(IN_GROUPS):
            if j < gend:
                return g

    with ExitStack() as ctx:
        xts = ctx.enter_context(nc.sbuf_tensor("xts", [KP, NJ * W], bf16))
        # staging keeps half1 in partitions 64:128 (no partition fold), so
        # output DMAs move 128 lines instead of 64 -- better engine pipelining
        st = ctx.enter_context(nc.sbuf_tensor("st", [2 * B, NJ, 96], bf16))
        # 2 quad-tensors of 4 PSUM banks each (slot = 512 fp32 = one bank);
        # extraction reads all slots of a quad in one 3D-AP op
        pps = [
            ctx.enter_context(nc.psum_tensor(f"pp{i}", [128, 4, 512], f32))
            for i in range(2)
        ]
        # one semaphore per input DMA (no semaphore reuse -> no guards needed)
        sem_in = [
            ctx.enter_context(nc.semaphore(f"s_in{i}"))
            for i in range(len(IN_GROUPS))
        ]
        sem_pe = ctx.enter_context(nc.semaphore("s_pe"))
        sem_dve = ctx.enter_context(nc.semaphore("s_dve"))
        sem_act = ctx.enter_context(nc.semaphore("s_act"))
        # qPoolDynamic is a single FIFO queue, so one semaphore (+16 per DMA)
        # suffices for the gpsimd-issued groups; the sync-issued groups land
        # on distinct qSPDynamicHW queues and get their own semaphores
        sem_out = ctx.enter_context(nc.semaphore("s_out"))
        sem_out2 = [
            ctx.enter_context(nc.semaphore(f"s_out2{i}")) for i in range(2)
        ]
        block_cm = nc.Block()
        block = block_cm.__enter__()

        # chunk ranges of the output groups (unit thresholds -> chunk bounds)
        out_bounds = [0] + [EX_UNITS[u - 1][1] for u in OUT_GROUPS_U]

        in_bounds = [0] + list(IN_GROUPS)

        @block.sync
        def _(sync):
            # all input groups from one engine: generation order == transfer
            # order, so group 0 always reaches the DMA engines first
            for g in range(len(IN_GROUPS)):
                sync.dma_start(
                    out=xts[:, W * in_bounds[g] : W * in_bounds[g + 1]],
                    in_=xwt_d[:, W * in_bounds[g] : W * in_bounds[g + 1]],
                ).then_inc(sem_in[g], 16)
            # last two output groups: HWDGE generation here overlaps gpsimd's
            for i, gi in enumerate((len(OUT_GROUPS_U) - 2, len(OUT_GROUPS_U) - 1)):
                u = OUT_GROUPS_U[gi]
                c0, c1 = out_bounds[gi], out_bounds[gi + 1]
                sync.wait_ge(sem_dve, u)
                sync.wait_ge(sem_act, u)
                sync.dma_start(
                    out=y_d[:, 96 * c0 : 96 * c1], in_=st[:, c0:c1, :]
                ).then_inc(sem_out2[i], 16)

        def unit_of(c):
            for u, (c0, c1) in enumerate(EX_UNITS):
                if c0 <= c < c1:
                    return u

        @block.tensor
        def _(tensor):
            reuse_th = 0
            for j in range(NJ):
                g = grp(j)
                if j == 0 or grp(j - 1) != g:
                    tensor.wait_ge(sem_in[g], 16)
                if j >= 8:
                    # PSUM slot of chunk j (bank j%8) is free once the unit
                    # holding chunk j-8 has BOTH half-copies retired
                    th = unit_of(j - 8) + 1
                    if th > reuse_th:
                        tensor.wait_ge(sem_dve, th)
                        tensor.wait_ge(sem_act, th)
                        reuse_th = th
                tensor.matmul(
                    pps[(j // 4) % 2].ap()[:, j % 4, 0:CT],
                    xts[:, W * j : W * j + 128],
                    xts[:, W * j + 128 : W * (j + 1)],
                    start=True,
                    stop=True,
                ).then_inc(sem_pe, 1)

        @block.vector
        def _(vector):
            # half0: outputs 0:96 of each chunk live in PSUM partitions 0:64;
            # one 3D-AP op extracts a whole unit (slots c0%4..)
            for u, (c0, c1) in enumerate(EX_UNITS):
                vector.wait_ge(sem_pe, c1)
                vector.tensor_copy(
                    st[0:B, c0:c1, 0:96],
                    pps[(c0 // 4) % 2].ap()[0:B, c0 % 4 : c0 % 4 + c1 - c0, 0:96],
                ).then_inc(sem_dve, 1)

        @block.scalar
        def _(scalar):
            # half1: outputs 96:192 of each chunk live in PSUM partitions 64:128
            for u, (c0, c1) in enumerate(EX_UNITS):
                scalar.wait_ge(sem_pe, c1)
                scalar.copy(
                    st[B : 2 * B, c0:c1, 0:96],
                    pps[(c0 // 4) % 2].ap()[B : 2 * B, c0 % 4 : c0 % 4 + c1 - c0, 96:CT],
                ).then_inc(sem_act, 1)

        @block.gpsimd
        def _(gpsimd):
            for gi in range(len(OUT_GROUPS_U) - 2):
                u = OUT_GROUPS_U[gi]
                c0, c1 = out_bounds[gi], out_bounds[gi + 1]
                gpsimd.wait_ge(sem_dve, u)
                gpsimd.wait_ge(sem_act, u)
                gpsimd.dma_start(
                    out=y_d[:, 96 * c0 : 96 * c1], in_=st[:, c0:c1, :]
                ).then_inc(sem_out, 16)
            gpsimd.wait_ge(sem_out, 16 * (len(OUT_GROUPS_U) - 2))
            for s in sem_out2:
                gpsimd.wait_ge(s, 16)

        block_cm.__exit__(None, None, None)  # all-engine exit barrier
        # no explicit semaphore-clear block: the nc.semaphore() context
        # managers emit clears for every sem at ExitStack exit, behind the
        # full-drain barrier above, so re-executing the NEFF is clean

    return nc


def run_sharded(inputs, trace=False):
    global _prog_cache
    x = np.ascontiguousarray(np.asarray(inputs["input"], dtype=np.float32))
    xwt = _prep_inputs(x, inputs["alpha"], inputs["beta"])
    if _prog_cache is None:
        _prog_cache = _build_program()
    nc = _prog_cache
    in_maps = [{"xwt": xwt[cc]} for cc in range(NCORES)]
    res = run_bass_kernel_spmd(nc, in_maps, list(range(NCORES)), trace=trace)
    shards = []
    for cc in range(NCORES):
        yd = res.results[cc]["y"].reshape(2, B, NJ, 96)
        yc = np.transpose(yd, (1, 2, 0, 3)).reshape(B, NJ * CT)
        shards.append(yc[:, :TCORE].astype(np.float32))
    y = np.concatenate(shards, axis=1)
    return y, res


def kernel(input, alpha, beta):
    y, _ = run_sharded({"input": input, "alpha": alpha, "beta": beta})
    return y


# revision 48
# speedup vs baseline: 1.1338x; 1.0077x over previous
"""Time-varying 33-tap FIR low-pass filter on 8 Trainium2 NeuronCores.

y[b,t] = sum_u filt[t,u] * x[b, t+u-16],  filt = host-computed windowed-sinc
bank (n,33) derived from scalars alpha/beta (tiny; O(n*33) host FLOPs).

Sharding: time dim split across the 8 cores (4096 t-columns each, all 64
batch rows).  Per core the banded matmul y = x @ W (contraction over input
time s) is tiled into 22 TensorE matmuls.  Each matmul packs TWO 128-sample
x-chunks, offset by 96 samples, side by side in the stationary operand
(K=128, M=128 = 2 halves x 64 batch).  The 96-offset makes every output
column's 33-tap band land entirely inside one half, so each PSUM column is
valid in exactly one 64-row half and the chunk serves 192 output columns
with no accumulation pass:

  lhsT[k, 64*h + b] = x[b, S + 96*h + k]           (S = core_t0 - 16 + 192*j)
  rhs [k, n]        = filt[S+16+n, u] at k = (n % 96) + u   (zeros elsewhere)
  psum[64*h(n) + b, n] = y[b, S+16+n],   h(n) = n // 96

Performance notes (38.9us baseline -> ~22us):
- all matmul operands bf16 (fp32 matmul runs 4 cycles/row on TRN2, bf16
  runs 1; input DMA bytes halve; rel err ~6e-3, gate is 2e-2)
- input laid out partition-major ([128, NJ*W]) and moved by a few LARGE
  grouped DMAs: the 16 per-core DMA engines have ~30ns/packet overhead, so
  long lines (group_chunks*640B) sustain ~280-306GB/s aggregate
- valid PSUM half-rows are extracted on-device -- DVE copies half0 (cols
  0:96 from partitions 0:64), the Activation engine copies half1 (cols
  96:192 from partitions 64:128) -- converting to bf16 into a compact
  [128, NJ*96] staging tile (half1 stays in partitions 64:128 so output
  DMAs move 128 lines, not 64), so the output DMA moves 1/4 of the
  baseline bytes; host unfolds the halves and upcasts to fp32
- extraction is batched via multi-bank 3D PSUM access patterns: quads up
  front for batching efficiency, pairs near the end so the tail's
  PSUM-reuse and output-DMA links are short
- output DMAs are queued just-in-time after the input stream drains (they
  share the DMA engines), split between gpsimd's SWDGE and the idle Sync
  engine's HWDGE so descriptor generations overlap
- the unused const-AP publish barrier in Bass.__init__ is skipped (~0.3us)
- no explicit semaphore-clear block: the nc.semaphore() context managers
  already emit clears at ExitStack exit behind the full-drain exit barrier
  (verified safe across NEFF re-executions), saving ~1us of teardown
Fixed costs bound further gains: ~6.7us NEFF preamble (runtime rendezvous,
iram loads, engine preambles), ~0.9us per-DMA completion-semaphore
latency, ~2us teardown (drains, exit barrier, scope-exit semaphore
clears).  An empty NEFF measures ~11.1us on this runtime.
"""

import sys
from contextlib import ExitStack

import numpy as np
import ml_dtypes

if "/opt/trn_rl_repo" not in sys.path:
    sys.path.insert(0, "/opt/trn_rl_repo")

from concourse import bass, mybir
from concourse.bass_utils import run_bass_kernel_spmd

N = 32768          # time length
B = 64             # batch
NCORES = 8
TCORE = N // NCORES            # 4096 output columns per core
CT = 192                       # output columns served per chunk
NJ = (TCORE + CT - 1) // CT    # 22 chunks per core (last one partial: 64 cols)
KP = 128                       # contraction rows per matmul
TAPS = 33
HALF = 16
W = 128 + CT                   # 320 columns per [stationary | moving] chunk

# input DMA groups (chunk boundaries).  Few LARGE groups: the 16 DMA engines
# run ~15-20GB/s per engine with ~30ns/packet overhead, so longer lines
# (group_chunks*640B) raise the per-engine rate.  A tiny first group starts
# compute early; a small last group keeps the final matmuls off a big
# trailing transfer's completion latency
IN_GROUPS = (4, 12, 18, NJ)
# extraction units: quads up front (efficient batching), pairs from chunk 12
# on so the tail's PSUM-reuse and output-issue links are short
EX_UNITS = ((0, 4), (4, 8), (8, 12), (12, 14), (14, 16), (16, 18), (18, 20), (20, NJ))
# output DMA groups as extraction-unit thresholds -> chunk ranges
# [0:4) [4:12) [12:16) [16:20) [20:22).  Queued just-in-time: transfers share
# the 16 DMA engines with the input stream, so the first (small) group rides
# right behind the input stream's tail and the rest chase extraction.  All
# but the last two go through gpsimd's SWDGE; the last two are issued by the
# (idle) Sync engine's HWDGE so their ~0.6us descriptor generations overlap
# gpsimd's instead of serializing.
OUT_GROUPS_U = (1, 3, 5, 7, 8)

_prog_cache = None


def _filters_np(alpha, beta):
    """Numpy port of reference._filters (returns the flipped bank)."""
    t = np.arange(N, dtype=np.float64)
    cutoff = (np.pi / 4.0 + float(alpha) * np.sin(float(beta) * t / 8000.0)) / (
        2.0 * np.pi
    )
    k = np.arange(TAPS, dtype=np.float64)
    window = 0.5 - 0.5 * np.cos(2.0 * np.pi * k / (TAPS - 1.0))
    tvec = np.arange(-HALF, HALF + 1, dtype=np.float64)
    arg = 2.0 * np.pi * cutoff[:, None] * tvec[None, :]
    safe = np.where(arg == 0.0, 1.0, arg)
    sinc = np.where(arg == 0.0, 1.0, np.sin(safe) / safe)
    f = 2.0 * cutoff[:, None] * window[None, :] * sinc
    f = f / f.sum(axis=-1, keepdims=True)
    return np.ascontiguousarray(f[:, ::-1]).astype(np.float32)


def _prep_inputs(x, alpha, beta):
    """Build per-core [KP, NJ*W] bf16 [stationary | banded-filter] tiles."""
    filt = _filters_np(alpha, beta)  # (N, 33)

    pad = 16 + N + 512
    xp = np.zeros((B, pad), dtype=np.float32)
    xp[:, 16 : 16 + N] = x
    xp = xp.astype(ml_dtypes.bfloat16)
    fp = np.zeros((N + 512, TAPS), dtype=ml_dtypes.bfloat16)
    fp[:N] = filt.astype(ml_dtypes.bfloat16)

    c = np.arange(NCORES)[:, None, None, None]
    j = np.arange(NJ)[None, :, None, None]
    h = np.arange(2)[None, None, :, None]
    k = np.arange(KP)[None, None, None, :]
    # global s = TCORE*c - 16 + CT*j + 96*h + k ; +16 shifts into xp coords
    sidx = TCORE * c + CT * j + 96 * h + k
    xw = xp[:, sidx]  # (B, NCORES, NJ, 2, KP)
    xw = np.ascontiguousarray(
        np.transpose(xw, (1, 2, 4, 3, 0)).reshape(NCORES, NJ, KP, 128)
    )

    u = np.arange(TAPS)[:, None]  # (33, 1)
    nn = np.arange(CT)[None, :]  # (1, 192)
    rows = (nn % 96) + u  # (33, 192) target partition rows
    cols = np.broadcast_to(nn, (TAPS, CT))
    tg = (
        TCORE * np.arange(NCORES)[:, None, None]
        + CT * np.arange(NJ)[None, :, None]
        + np.arange(CT)[None, None, :]
    )  # (NCORES, NJ, 192) global output t per column
    vals = np.transpose(fp[tg], (0, 1, 3, 2))  # (NCORES, NJ, 33, 192)
    wt = np.zeros((NCORES, NJ, KP, CT), dtype=ml_dtypes.bfloat16)
    wt[:, :, rows, cols] = vals

    # one combined [stationary | moving] tile per chunk, then partition-major
    # ([KP, NJ, W]) so grouped input DMAs move long contiguous lines
    xwt = np.concatenate([xw, wt], axis=3)  # (NCORES, NJ, KP, W)
    xwt = np.transpose(xwt, (0, 2, 1, 3)).reshape(NCORES, KP, NJ * W)
    return np.ascontiguousarray(xwt)


def _build_program():
    """Raw Bass (no Tile): walrus permits a single sync-wait slot per Matmult
    and per DMA descriptor, so waits are emitted as standalone EventSemaphore
    instructions on each engine's queue instead."""
    # Skip the const-AP publish barrier at the tail of Bass.__init__: this
    # kernel never reads const_aps (scalar Copy keeps a float bias), the NRT
    # pseudo-barrier earlier in init already rendezvoused the engines, and
    # per-sem waits order everything else.  Saves ~0.3us of preamble.
    orig_aeb = bass.Bass.all_engine_barrier
    bass.Bass.all_engine_barrier = lambda self, *, sem_only=False: None
    try:
        nc = bass.Bass(trn_type="TRN2", debug=False)
    finally:
        bass.Bass.all_engine_barrier = orig_aeb
    f32 = mybir.dt.float32
    bf16 = mybir.dt.bfloat16
    xwt_d = nc.dram_tensor("xwt", [KP, NJ * W], bf16, kind="ExternalInput").ap()
    y_d = nc.dram_tensor("y", [2 * B, NJ * 96], bf16, kind="ExternalOutput").ap()

    def grp(j):
        """Input-group index of chunk j."""
        for g, gend in enumerate(IN_GROUPS):
            if j < gend:
                return g

    with ExitStack() as ctx:
        xts = ctx.enter_context(nc.sbuf_tensor("xts", [KP, NJ * W], bf16))
        # staging keeps half1 in partitions 64:128 (no partition fold), so
        # output DMAs move 128 lines instead of 64 -- better engine pipelining
        st = ctx.enter_context(nc.sbuf_tensor("st", [2 * B, NJ, 96], bf16))
        # 2 quad-tensors of 4 PSUM banks each (slot = 512 fp32 = one bank);
        # extraction reads all slots of a quad in one 3D-AP op
        pps = [
            ctx.enter_context(nc.psum_tensor(f"pp{i}", [128, 4, 512], f32))
            for i in range(2)
        ]
        # one semaphore per input DMA (no semaphore reuse -> no guards needed)
        sem_in = [
            ctx.enter_context(nc.semaphore(f"s_in{i}"))
            for i in range(len(IN_GROUPS))
        ]
        sem_pe = ctx.enter_context(nc.semaphore("s_pe"))
        sem_dve = ctx.enter_context(nc.semaphore("s_dve"))
        sem_act = ctx.enter_context(nc.semaphore("s_act"))
        # gpsimd's SWDGE serializes gen->transfer per DMA, so it gets only the
        # first output group; the rest go through Sync's HWDGE whose 4 round-
        # robin queues overlap transfers -- one semaphore per DMA
        sem_out = ctx.enter_context(nc.semaphore("s_out"))
        sem_out2 = [
            ctx.enter_context(nc.semaphore(f"s_out2{i}"))
            for i in range(len(OUT_GROUPS_U) - 1)
        ]
        block_cm = nc.Block()
        block = block_cm.__enter__()

        # chunk ranges of the output groups (unit thresholds -> chunk bounds)
        out_bounds = [0] + [EX_UNITS[u - 1][1] for u in OUT_GROUPS_U]

        in_bounds = [0] + list(IN_GROUPS)

        @block.sync
        def _(sync):
            # all input groups from one engine: generation order == transfer
            # order, so group 0 always reaches the DMA engines first
            for g in range(len(IN_GROUPS)):
                sync.dma_start(
                    out=xts[:, W * in_bounds[g] : W * in_bounds[g + 1]],
                    in_=xwt_d[:, W * in_bounds[g] : W * in_bounds[g + 1]],
                ).then_inc(sem_in[g], 16)
            # all output groups but the first: HWDGE gens pipeline on the SP
            # SEQ while the transfers overlap across the 4 HW queues
            for i, gi in enumerate(range(1, len(OUT_GROUPS_U))):
                u = OUT_GROUPS_U[gi]
                c0, c1 = out_bounds[gi], out_bounds[gi + 1]
                sync.wait_ge(sem_dve, u)
                sync.wait_ge(sem_act, u)
                sync.dma_start(
                    out=y_d[:, 96 * c0 : 96 * c1], in_=st[:, c0:c1, :]
                ).then_inc(sem_out2[i], 16)

        def unit_of(c):
            for u, (c0, c1) in enumerate(EX_UNITS):
                if c0 <= c < c1:
                    return u

        @block.tensor
        def _(tensor):
            reuse_th = 0
            for j in range(NJ):
                g = grp(j)
                if j == 0 or grp(j - 1) != g:
                    tensor.wait_ge(sem_in[g], 16)
                if j >= 8:
                    # PSUM slot of chunk j (bank j%8) is free once the unit
                    # holding chunk j-8 has BOTH half-copies retired
                    th = unit_of(j - 8) + 1
                    if th > reuse_th:
                        tensor.wait_ge(sem_dve, th)
                        tensor.wait_ge(sem_act, th)
                        reuse_th = th
                tensor.matmul(
                    pps[(j // 4) % 2].ap()[:, j % 4, 0:CT],
                    xts[:, W * j : W * j + 128],
                    xts[:, W * j + 128 : W * (j + 1)],
                    start=True,
                    stop=True,
                ).then_inc(sem_pe, 1)

        @block.vector
        def _(vector):
            # half0: outputs 0:96 of each chunk live in PSUM partitions 0:64;
            # one 3D-AP op extracts a whole unit (slots c0%4..)
            for u, (c0, c1) in enumerate(EX_UNITS):
                vector.wait_ge(sem_pe, c1)
                vector.tensor_copy(
                    st[0:B, c0:c1, 0:96],
                    pps[(c0 // 4) % 2].ap()[0:B, c0 % 4 : c0 % 4 + c1 - c0, 0:96],
                ).then_inc(sem_dve, 1)

        @block.scalar
        def _(scalar):
            # half1: outputs 96:192 of each chunk live in PSUM partitions 64:128
            for u, (c0, c1) in enumerate(EX_UNITS):
                scalar.wait_ge(sem_pe, c1)
                scalar.copy(
                    st[B : 2 * B, c0:c1, 0:96],
                    pps[(c0 // 4) % 2].ap()[B : 2 * B, c0 % 4 : c0 % 4 + c1 - c0, 96:CT],
                ).then_inc(sem_act, 1)

        @block.gpsimd
        def _(gpsimd):
            for gi in (0,):
                u = OUT_GROUPS_U[gi]
                c0, c1 = out_bounds[gi], out_bounds[gi + 1]
                gpsimd.wait_ge(sem_dve, u)
                gpsimd.wait_ge(sem_act, u)
                gpsimd.dma_start(
                    out=y_d[:, 96 * c0 : 96 * c1], in_=st[:, c0:c1, :]
                ).then_inc(sem_out, 16)
            gpsimd.wait_ge(sem_out, 16)
            for s in sem_out2:
                gpsimd.wait_ge(s, 16)

        block_cm.__exit__(None, None, None)  # all-engine exit barrier
        # no explicit semaphore-clear block: the nc.semaphore() context
        # managers emit clears for every sem at ExitStack exit, behind the
        # full-drain barrier above, so re-executing the NEFF is clean

    return nc


def run_sharded(inputs, trace=False):
    global _prog_cache
    x = np.ascontiguousarray(np.asarray(inputs["input"], dtype=np.float32))
    xwt = _prep_inputs(x, inputs["alpha"], inputs["beta"])
    if _prog_cache is None:
        _prog_cache = _build_program()
    nc = _prog_cache
    in_maps = [{"xwt": xwt[cc]} for cc in range(NCORES)]
    res = run_bass_kernel_spmd(nc, in_maps, list(range(NCORES)), trace=trace)
    shards = []
    for cc in range(NCORES):
        yd = res.results[cc]["y"].reshape(2, B, NJ, 96)
        yc = np.transpose(yd, (1, 2, 0, 3)).reshape(B, NJ * CT)
        shards.append(yc[:, :TCORE].astype(np.float32))
    y = np.concatenate(shards, axis=1)
    return y, res


def kernel(input, alpha, beta):
    y, _ = run_sharded({"input": input, "alpha": alpha, "beta": beta})
    return y


# revision 49
# speedup vs baseline: 1.1361x; 1.0020x over previous
"""Time-varying 33-tap FIR low-pass filter on 8 Trainium2 NeuronCores.

y[b,t] = sum_u filt[t,u] * x[b, t+u-16],  filt = host-computed windowed-sinc
bank (n,33) derived from scalars alpha/beta (tiny; O(n*33) host FLOPs).

Sharding: time dim split across the 8 cores (4096 t-columns each, all 64
batch rows).  Per core the banded matmul y = x @ W (contraction over input
time s) is tiled into 22 TensorE matmuls.  Each matmul packs TWO 128-sample
x-chunks, offset by 96 samples, side by side in the stationary operand
(K=128, M=128 = 2 halves x 64 batch).  The 96-offset makes every output
column's 33-tap band land entirely inside one half, so each PSUM column is
valid in exactly one 64-row half and the chunk serves 192 output columns
with no accumulation pass:

  lhsT[k, 64*h + b] = x[b, S + 96*h + k]           (S = core_t0 - 16 + 192*j)
  rhs [k, n]        = filt[S+16+n, u] at k = (n % 96) + u   (zeros elsewhere)
  psum[64*h(n) + b, n] = y[b, S+16+n],   h(n) = n // 96

Performance notes (38.9us baseline -> ~22us):
- all matmul operands bf16 (fp32 matmul runs 4 cycles/row on TRN2, bf16
  runs 1; input DMA bytes halve; rel err ~6e-3, gate is 2e-2)
- input laid out partition-major ([128, NJ*W]) and moved by a few LARGE
  grouped DMAs: the 16 per-core DMA engines have ~30ns/packet overhead, so
  long lines (group_chunks*640B) sustain ~280-306GB/s aggregate
- valid PSUM half-rows are extracted on-device -- DVE copies half0 (cols
  0:96 from partitions 0:64), the Activation engine copies half1 (cols
  96:192 from partitions 64:128) -- converting to bf16 into a compact
  [128, NJ*96] staging tile (half1 stays in partitions 64:128 so output
  DMAs move 128 lines, not 64), so the output DMA moves 1/4 of the
  baseline bytes; host unfolds the halves and upcasts to fp32
- extraction is batched via multi-bank 3D PSUM access patterns: quads up
  front for batching efficiency, pairs near the end so the tail's
  PSUM-reuse and output-DMA links are short
- output DMAs are queued just-in-time after the input stream drains (they
  share the DMA engines); gpsimd's SWDGE serializes gen->transfer per DMA,
  so only the first group goes there -- the rest are issued by the idle
  Sync engine's HWDGE, whose 4 round-robin queues overlap transfers
- the unused const-AP publish barrier in Bass.__init__ is skipped (~0.3us)
- no explicit semaphore-clear block: the nc.semaphore() context managers
  already emit clears at ExitStack exit behind the full-drain exit barrier
  (verified safe across NEFF re-executions), saving ~1us of teardown
Fixed costs bound further gains: ~6.7us NEFF preamble (runtime rendezvous,
iram loads, engine preambles), ~0.9us per-DMA completion-semaphore
latency, ~2us teardown (drains, exit barrier, scope-exit semaphore
clears).  An empty NEFF measures ~11.1us on this runtime.
"""

import sys
from contextlib import ExitStack

import numpy as np
import ml_dtypes

if "/opt/trn_rl_repo" not in sys.path:
    sys.path.insert(0, "/opt/trn_rl_repo")

from concourse import bass, mybir
from concourse.bass_utils import run_bass_kernel_spmd

N = 32768          # time length
B = 64             # batch
NCORES = 8
TCORE = N // NCORES            # 4096 output columns per core
CT = 192                       # output columns served per chunk
NJ = (TCORE + CT - 1) // CT    # 22 chunks per core (last one partial: 64 cols)
KP = 128                       # contraction rows per matmul
TAPS = 33
HALF = 16
W = 128 + CT                   # 320 columns per [stationary | moving] chunk

# input DMA groups (chunk boundaries).  Few LARGE groups: the 16 DMA engines
# run ~15-20GB/s per engine with ~30ns/packet overhead, so longer lines
# (group_chunks*640B) raise the per-engine rate.  A tiny first group starts
# compute early; a small last group keeps the final matmuls off a big
# trailing transfer's completion latency
IN_GROUPS = (4, 12, 18, NJ)
# extraction units: quads up front (efficient batching), pairs from chunk 12
# on so the tail's PSUM-reuse and output-issue links are short
EX_UNITS = ((0, 4), (4, 8), (8, 12), (12, 14), (14, 16), (16, 18), (18, 20), (20, NJ))
# output DMA groups as extraction-unit thresholds -> chunk ranges
# [0:4) [4:12) [12:16) [16:20) [20:22).  Queued just-in-time: transfers share
# the 16 DMA engines with the input stream, so the first (small) group rides
# right behind the input stream's tail and the rest chase extraction.  The
# first goes through gpsimd's SWDGE (which serializes gen->transfer per
# DMA); the rest are issued by the (idle) Sync engine's HWDGE whose 4
# round-robin queues pipeline generations AND overlap transfers.
OUT_GROUPS_U = (1, 3, 5, 7, 8)

_prog_cache = None


def _filters_np(alpha, beta):
    """Numpy port of reference._filters (returns the flipped bank)."""
    t = np.arange(N, dtype=np.float64)
    cutoff = (np.pi / 4.0 + float(alpha) * np.sin(float(beta) * t / 8000.0)) / (
        2.0 * np.pi
    )
    k = np.arange(TAPS, dtype=np.float64)
    window = 0.5 - 0.5 * np.cos(2.0 * np.pi * k / (TAPS - 1.0))
    tvec = np.arange(-HALF, HALF + 1, dtype=np.float64)
    arg = 2.0 * np.pi * cutoff[:, None] * tvec[None, :]
    safe = np.where(arg == 0.0, 1.0, arg)
    sinc = np.where(arg == 0.0, 1.0, np.sin(safe) / safe)
    f = 2.0 * cutoff[:, None] * window[None, :] * sinc
    f = f / f.sum(axis=-1, keepdims=True)
    return np.ascontiguousarray(f[:, ::-1]).astype(np.float32)


def _prep_inputs(x, alpha, beta):
    """Build per-core [KP, NJ*W] bf16 [stationary | banded-filter] tiles."""
    filt = _filters_np(alpha, beta)  # (N, 33)

    pad = 16 + N + 512
    xp = np.zeros((B, pad), dtype=np.float32)
    xp[:, 16 : 16 + N] = x
    xp = xp.astype(ml_dtypes.bfloat16)
    fp = np.zeros((N + 512, TAPS), dtype=ml_dtypes.bfloat16)
    fp[:N] = filt.astype(ml_dtypes.bfloat16)

    c = np.arange(NCORES)[:, None, None, None]
    j = np.arange(NJ)[None, :, None, None]
    h = np.arange(2)[None, None, :, None]
    k = np.arange(KP)[None, None, None, :]
    # global s = TCORE*c - 16 + CT*j + 96*h + k ; +16 shifts into xp coords
    sidx = TCORE * c + CT * j + 96 * h + k
    xw = xp[:, sidx]  # (B, NCORES, NJ, 2, KP)
    xw = np.ascontiguousarray(
        np.transpose(xw, (1, 2, 4, 3, 0)).reshape(NCORES, NJ, KP, 128)
    )

    u = np.arange(TAPS)[:, None]  # (33, 1)
    nn = np.arange(CT)[None, :]  # (1, 192)
    rows = (nn % 96) + u  # (33, 192) target partition rows
    cols = np.broadcast_to(nn, (TAPS, CT))
    tg = (
        TCORE * np.arange(NCORES)[:, None, None]
        + CT * np.arange(NJ)[None, :, None]
        + np.arange(CT)[None, None, :]
    )  # (NCORES, NJ, 192) global output t per column
    vals = np.transpose(fp[tg], (0, 1, 3, 2))  # (NCORES, NJ, 33, 192)
    wt = np.zeros((NCORES, NJ, KP, CT), dtype=ml_dtypes.bfloat16)
    wt[:, :, rows, cols] = vals

    # one combined [stationary | moving] tile per chunk, then partition-major
    # ([KP, NJ, W]) so grouped input DMAs move long contiguous lines
    xwt = np.concatenate([xw, wt], axis=3)  # (NCORES, NJ, KP, W)
    xwt = np.transpose(xwt, (0, 2, 1, 3)).reshape(NCORES, KP, NJ * W)
    return np.ascontiguousarray(xwt)


def _build_program():
    """Raw Bass (no Tile): walrus permits a single sync-wait slot per Matmult
    and per DMA descriptor, so waits are emitted as standalone EventSemaphore
    instructions on each engine's queue instead."""
    # Skip the const-AP publish barrier at the tail of Bass.__init__: this
    # kernel never reads const_aps (scalar Copy keeps a float bias), the NRT
    # pseudo-barrier earlier in init already rendezvoused the engines, and
    # per-sem waits order everything else.  Saves ~0.3us of preamble.
    orig_aeb = bass.Bass.all_engine_barrier
    bass.Bass.all_engine_barrier = lambda self, *, sem_only=False: None
    try:
        nc = bass.Bass(trn_type="TRN2", debug=False)
    finally:
        bass.Bass.all_engine_barrier = orig_aeb
    f32 = mybir.dt.float32
    bf16 = mybir.dt.bfloat16
    xwt_d = nc.dram_tensor("xwt", [KP, NJ * W], bf16, kind="ExternalInput").ap()
    y_d = nc.dram_tensor("y", [2 * B, NJ * 96], bf16, kind="ExternalOutput").ap()

    def grp(j):
        """Input-group index of chunk j."""
        for g, gend in enumerate(IN_GROUPS):
            if j < gend:
                return g

    with ExitStack() as ctx:
        xts = ctx.enter_context(nc.sbuf_tensor("xts", [KP, NJ * W], bf16))
        # staging keeps half1 in partitions 64:128 (no partition fold), so
        # output DMAs move 128 lines instead of 64 -- better engine pipelining
        st = ctx.enter_context(nc.sbuf_tensor("st", [2 * B, NJ, 96], bf16))
        # 2 quad-tensors of 4 PSUM banks each (slot = 512 fp32 = one bank);
        # extraction reads all slots of a quad in one 3D-AP op
        pps = [
            ctx.enter_context(nc.psum_tensor(f"pp{i}", [128, 4, 512], f32))
            for i in range(2)
        ]
        # one semaphore per input DMA (no semaphore reuse -> no guards needed)
        sem_in = [
            ctx.enter_context(nc.semaphore(f"s_in{i}"))
            for i in range(len(IN_GROUPS))
        ]
        sem_pe = ctx.enter_context(nc.semaphore("s_pe"))
        sem_dve = ctx.enter_context(nc.semaphore("s_dve"))
        sem_act = ctx.enter_context(nc.semaphore("s_act"))
        # gpsimd's SWDGE serializes gen->transfer per DMA, so it gets only the
        # first output group; the rest go through Sync's HWDGE whose 4 round-
        # robin queues overlap transfers -- one semaphore per DMA
        sem_out = ctx.enter_context(nc.semaphore("s_out"))
        sem_out2 = [
            ctx.enter_context(nc.semaphore(f"s_out2{i}"))
            for i in range(len(OUT_GROUPS_U) - 1)
        ]
        block_cm = nc.Block()
        block = block_cm.__enter__()

        # chunk ranges of the output groups (unit thresholds -> chunk bounds)
        out_bounds = [0] + [EX_UNITS[u - 1][1] for u in OUT_GROUPS_U]

        in_bounds = [0] + list(IN_GROUPS)

        @block.sync
        def _(sync):
            # all input groups from one engine: generation order == transfer
            # order, so group 0 always reaches the DMA engines first
            for g in range(len(IN_GROUPS)):
                sync.dma_start(
                    out=xts[:, W * in_bounds[g] : W * in_bounds[g + 1]],
                    in_=xwt_d[:, W * in_bounds[g] : W * in_bounds[g + 1]],
                ).then_inc(sem_in[g], 16)
            # all output groups but the first: HWDGE gens pipeline on the SP
            # SEQ while the transfers overlap across the 4 HW queues
            for i, gi in enumerate(range(1, len(OUT_GROUPS_U))):
                u = OUT_GROUPS_U[gi]
                c0, c1 = out_bounds[gi], out_bounds[gi + 1]
                sync.wait_ge(sem_dve, u)
                sync.wait_ge(sem_act, u)
                sync.dma_start(
                    out=y_d[:, 96 * c0 : 96 * c1], in_=st[:, c0:c1, :]
                ).then_inc(sem_out2[i], 16)

        def unit_of(c):
            for u, (c0, c1) in enumerate(EX_UNITS):
                if c0 <= c < c1:
                    return u

        @block.tensor
        def _(tensor):
            reuse_th = 0
            for j in range(NJ):
                g = grp(j)
                if j == 0 or grp(j - 1) != g:
                    tensor.wait_ge(sem_in[g], 16)
                if j >= 8:
                    # PSUM slot of chunk j (bank j%8) is free once the unit
                    # holding chunk j-8 has BOTH half-copies retired
                    th = unit_of(j - 8) + 1
                    if th > reuse_th:
                        tensor.wait_ge(sem_dve, th)
                        tensor.wait_ge(sem_act, th)
                        reuse_th = th
                tensor.matmul(
                    pps[(j // 4) % 2].ap()[:, j % 4, 0:CT],
                    xts[:, W * j : W * j + 128],
                    xts[:, W * j + 128 : W * (j + 1)],
                    start=True,
                    stop=True,
                ).then_inc(sem_pe, 1)

        @block.vector
        def _(vector):
            # half0: outputs 0:96 of each chunk live in PSUM partitions 0:64;
            # one 3D-AP op extracts a whole unit (slots c0%4..)
            for u, (c0, c1) in enumerate(EX_UNITS):
                vector.wait_ge(sem_pe, c1)
                vector.tensor_copy(
                    st[0:B, c0:c1, 0:96],
                    pps[(c0 // 4) % 2].ap()[0:B, c0 % 4 : c0 % 4 + c1 - c0, 0:96],
                ).then_inc(sem_dve, 1)

        @block.scalar
        def _(scalar):
            # half1: outputs 96:192 of each chunk live in PSUM partitions 64:128
            for u, (c0, c1) in enumerate(EX_UNITS):
                scalar.wait_ge(sem_pe, c1)
                scalar.copy(
                    st[B : 2 * B, c0:c1, 0:96],
                    pps[(c0 // 4) % 2].ap()[B : 2 * B, c0 % 4 : c0 % 4 + c1 - c0, 96:CT],
                ).then_inc(sem_act, 1)

        @block.gpsimd
        def _(gpsimd):
            for gi in (0,):
                u = OUT_GROUPS_U[gi]
                c0, c1 = out_bounds[gi], out_bounds[gi + 1]
                gpsimd.wait_ge(sem_dve, u)
                gpsimd.wait_ge(sem_act, u)
                gpsimd.dma_start(
                    out=y_d[:, 96 * c0 : 96 * c1], in_=st[:, c0:c1, :]
                ).then_inc(sem_out, 16)
            gpsimd.wait_ge(sem_out, 16)
            for s in sem_out2:
                gpsimd.wait_ge(s, 16)

        block_cm.__exit__(None, None, None)  # all-engine exit barrier
        # no explicit semaphore-clear block: the nc.semaphore() context
        # managers emit clears for every sem at ExitStack exit, behind the
        # full-drain barrier above, so re-executing the NEFF is clean

    return nc


def run_sharded(inputs, trace=False):
    global _prog_cache
    x = np.ascontiguousarray(np.asarray(inputs["input"], dtype=np.float32))
    xwt = _prep_inputs(x, inputs["alpha"], inputs["beta"])
    if _prog_cache is None:
        _prog_cache = _build_program()
    nc = _prog_cache
    in_maps = [{"xwt": xwt[cc]} for cc in range(NCORES)]
    res = run_bass_kernel_spmd(nc, in_maps, list(range(NCORES)), trace=trace)
    shards = []
    for cc in range(NCORES):
        yd = res.results[cc]["y"].reshape(2, B, NJ, 96)
        yc = np.transpose(yd, (1, 2, 0, 3)).reshape(B, NJ * CT)
        shards.append(yc[:, :TCORE].astype(np.float32))
    y = np.concatenate(shards, axis=1)
    return y, res


def kernel(input, alpha, beta):
    y, _ = run_sharded({"input": input, "alpha": alpha, "beta": beta})
    return y
